# revision 20
# baseline (speedup 1.0000x reference)
"""Multi-head causal self-attention (B=4, S=2048, H=16, D=128) on 8 TRN2 cores.

Sharding: core c = (batch b = c//2, head-group g = c%2 of 8 heads).
Each core computes Q/K projections (bf16) for its 8 heads, V, causal
attention, and the partial output projection. Host sums the two head-group
partials per batch and adds bo + bv@Wo (see bias folds below).

Bias folds (exact):
 - K bias is dropped: softmax_k[(K+bk)·(Q+bq)] == softmax_k[K·(Q+bq)]
   (terms constant along k cancel).
 - V bias and bo are folded into the host gather: normalized probs sum to
   1, so ctx_norm = ctx0/r + bv, whose projection is the constant row
   vector bv@Wo added host-side.

Structure (all matmul operands bf16; PSUM fp32):
 - kT/qT per head: [128 d, 2048 s]; V: 16 s-chunks of [128 s, 8*128 d].
 - k-tiles are processed in PAIRS: two score matmuls fill the two banks of
   one [128,1024] PSUM tile, ONE ACT exp covers both (amortizes the
   per-instruction overhead), writing a [128,1024] bf16 probs tile.
 - Softmax denominators WITHOUT per-tile ones-matmuls: a per-(head,qblock)
   running sum pacc [128,1024] accumulates the probs tiles on DVE (even
   heads) / GPSIMD (odd heads) -- engines that are otherwise idle -- and a
   single pair of E8 one-hot matmuls per (head,qblock) reduces pacc into
   the shared [8,512] row PSUM tile (head h lands on partition h). One DVE
   reciprocal per qblock serves all 8 heads.
 - Causal diagonal: triangular streaming (q starts at the k-tile diagonal)
   + one [128,128] bf16 mask multiply per diagonal subtile. Diagonal
   subtiles are exp-merged as (ki0,ki2) and (ki1,ki3) pairs.
 - Normalization: broadcast 1/r via one-hot-row E2 matmul, one DVE
   multiply, out-projection accumulates all 8 heads in one PSUM bank.
 - The per-qb epilogue is EMITTED one head-pair into the next q-block so
   the PE never idles at the boundary (HAM stays warm); ctx staging is
   double-buffered by qb parity to keep that legal.

q-blocks run in ASCENDING size order: the small early blocks overlap the
phase-1 drain, every deferred epilogue lands in a BIGGER next block, and
the dense qb=3 stream finishes the kernel with the PE warm."""

import os
import sys

import numpy as np

NUM_HEADS = 16
D = 128
B = 4
S = 2048
HPC = 8  # heads per core
N_CORES = 8
SCALE = 1.0 / np.sqrt(128.0)

_CACHE = {}


def _import_concourse():
    if "/opt/trn_rl_repo" not in sys.path and os.path.isdir("/opt/trn_rl_repo"):
        sys.path.insert(0, "/opt/trn_rl_repo")


def _build_nc():
    _import_concourse()
    from contextlib import ExitStack

    import concourse.mybir as mybir
    import concourse.tile as tile
    from concourse import bacc

    F32 = mybir.dt.float32
    BF16 = mybir.dt.bfloat16
    EXP = mybir.ActivationFunctionType.Exp

    nc = bacc.Bacc(trn_type="TRN2", target_bir_lowering=False, debug=False)

    xt_d = nc.dram_tensor("xt", [128, S], BF16, kind="ExternalInput").ap()
    wq_d = nc.dram_tensor("wq", [128, HPC * 128], BF16, kind="ExternalInput").ap()
    wk_d = nc.dram_tensor("wk", [128, HPC * 128], BF16, kind="ExternalInput").ap()
    wv_d = nc.dram_tensor("wv", [128, HPC * 128], BF16, kind="ExternalInput").ap()
    wo_d = nc.dram_tensor("wo", [128, HPC * 128], BF16, kind="ExternalInput").ap()
    bq_d = nc.dram_tensor("bqc", [128, HPC], F32, kind="ExternalInput").ap()
    tri_d = nc.dram_tensor("tri", [128, 128], BF16, kind="ExternalInput").ap()
    e8_d = nc.dram_tensor("e8", [128, HPC * HPC], BF16, kind="ExternalInput").ap()
    e2_d = nc.dram_tensor("e2", [HPC, HPC * 128], BF16, kind="ExternalInput").ap()
    out_d = nc.dram_tensor("out_t", [128, S], F32, kind="ExternalOutput").ap()

    with ExitStack() as ctx:
        ctx.enter_context(
            nc.allow_low_precision(reason="bf16 attention, tol 2e-2 rel")
        )
        tc = ctx.enter_context(tile.TileContext(nc))
        sb = ctx.enter_context(tc.tile_pool(name="sb", bufs=1))
        ptp = ctx.enter_context(tc.tile_pool(name="ptp", bufs=8))
        pap = ctx.enter_context(tc.tile_pool(name="pap", bufs=6))
        csp = ctx.enter_context(tc.tile_pool(name="csp", bufs=3))
        rcp = ctx.enter_context(tc.tile_pool(name="rcp", bufs=2))
        ps = ctx.enter_context(tc.tile_pool(name="ps", bufs=2, space="PSUM"))
        psc = ctx.enter_context(tc.tile_pool(name="psc", bufs=2, space="PSUM"))
        psr = ctx.enter_context(tc.tile_pool(name="psr", bufs=1, space="PSUM"))
        pso = ctx.enter_context(tc.tile_pool(name="pso", bufs=1, space="PSUM"))

        def load(name, dram_ap, shape, dt):
            t = sb.tile(shape, dt, tag=name, name=name)
            nc.sync.dma_start(t[:], dram_ap[:])
            return t

        xt = load("xt", xt_d, [128, S], BF16)
        wq = load("wq", wq_d, [128, HPC * 128], BF16)
        wk = load("wk", wk_d, [128, HPC * 128], BF16)
        wv = load("wv", wv_d, [128, HPC * 128], BF16)
        wo = load("wo", wo_d, [128, HPC * 128], BF16)
        bqc = load("bqc", bq_d, [128, HPC], F32)
        tri = load("tri", tri_d, [128, 128], BF16)
        e8 = load("e8", e8_d, [128, HPC * HPC], BF16)
        e2 = load("e2", e2_d, [HPC, HPC * 128], BF16)

        out_acc = sb.tile([128, S], F32, tag="out_acc")
        # ctx staging double-buffered by qb parity (the deferred epilogue of
        # qb reads while qb-1's pairs write the other half)
        ctx_all = sb.tile([128, 2 * HPC * 512], BF16, tag="ctx_all")

        # ---- phase 1: projections ----
        kT = [sb.tile([128, S], BF16, tag=f"kT{h}", name=f"kT{h}") for h in range(HPC)]
        qT = [sb.tile([128, S], BF16, tag=f"qT{h}", name=f"qT{h}") for h in range(HPC)]
        vsb = [
            sb.tile([128, HPC * 128], BF16, tag=f"v{st}", name=f"v{st}")
            for st in range(16)
        ]

        def proj_head(h):
            # 1024-wide PSUM staging (2 banks, 2 matmuls) with one wide
            # drain copy -- K on DVE, Q on ACT (bias fused) -- so the drain
            # keeps pace with the PE and HAM warms up early
            hs = slice(h * 128, (h + 1) * 128)
            for s2 in range(2):
                sl = slice(s2 * 1024, (s2 + 1) * 1024)
                psK = ps.tile([128, 1024], F32, tag="ps", name="psK")
                for half in range(2):
                    xsl = slice(s2 * 1024 + half * 512, s2 * 1024 + (half + 1) * 512)
                    nc.tensor.matmul(
                        psK[:, half * 512 : (half + 1) * 512], wk[:, hs], xt[:, xsl],
                        start=True, stop=True,
                    )
                nc.vector.tensor_copy(kT[h][:, sl], psK[:])
                psQ = ps.tile([128, 1024], F32, tag="ps", name="psQ")
                for half in range(2):
                    xsl = slice(s2 * 1024 + half * 512, s2 * 1024 + (half + 1) * 512)
                    nc.tensor.matmul(
                        psQ[:, half * 512 : (half + 1) * 512], wq[:, hs], xt[:, xsl],
                        start=True, stop=True,
                    )
                nc.scalar.activation(
                    qT[h][:, sl], psQ[:],
                    mybir.ActivationFunctionType.Identity,
                    bias=bqc[:, h : h + 1], scale=1.0,
                )

        def proj_v():
            for st in range(16):
                xsl = slice(st * 128, (st + 1) * 128)
                psV = ps.tile([128, 1024], F32, tag="ps", name="psV")
                for j in range(2):
                    wsl = slice(j * 512, (j + 1) * 512)
                    nc.tensor.matmul(
                        psV[:, wsl], xt[:, xsl], wv[:, wsl], start=True, stop=True
                    )
                if st % 2 == 0:
                    nc.vector.tensor_copy(vsb[st][:], psV[:])
                else:
                    nc.scalar.copy(vsb[st][:], psV[:])

        proj_head(0)
        proj_head(1)
        proj_v()
        for h in range(2, HPC):
            proj_head(h)

        # ---- phase 2: attention, q-blocks descending ----
        pending_epilogue = [None]

        def flush_epilogue():
            if pending_epilogue[0] is not None:
                pending_epilogue[0]()
                pending_epilogue[0] = None

        def acc_eng(h):
            # pacc/mask engine: DVE for even heads, GPSIMD for odd
            return nc.vector if h % 2 == 0 else nc.gpsimd

        for qb in (0, 1, 2, 3):
            nkt = 4 * (qb + 1)
            qbase = qb * 512
            row_ps = psr.tile([HPC, 512], F32, tag="row", name="row_ps")
            out_ps = pso.tile([128, 512], F32, tag="o", name="out_ps")
            row_started = [False]
            for hp in range(4):
                h0, h1 = 2 * hp, 2 * hp + 1
                cps = [
                    psc.tile([128, 512], F32, tag="ctx", name="ctx_ps")
                    for _ in range(2)
                ]
                pacc = [
                    pap.tile([128, 1024], BF16, tag="pacc", name="pacc")
                    for _ in range(2)
                ]
                # non-diagonal k-tile pairs
                for p in range((nkt - 4) // 2):
                    kt0, kt1 = 2 * p, 2 * p + 1
                    T = [None, None]
                    for j, h in ((0, h0), (1, h1)):
                        sp = ps.tile([128, 1024], F32, tag="ps", name="s_ps")
                        nc.tensor.matmul(
                            sp[:, 0:512],
                            kT[h][:, kt0 * 128 : (kt0 + 1) * 128],
                            qT[h][:, qbase : qbase + 512],
                            start=True, stop=True,
                        )
                        nc.tensor.matmul(
                            sp[:, 512:1024],
                            kT[h][:, kt1 * 128 : (kt1 + 1) * 128],
                            qT[h][:, qbase : qbase + 512],
                            start=True, stop=True,
                        )
                        T[j] = ptp.tile([128, 1024], BF16, tag="pT", name="pT")
                        nc.scalar.activation(T[j][:], sp[:], EXP, scale=float(SCALE))
                    for j, h in ((0, h0), (1, h1)):
                        eng = acc_eng(h)
                        if p == 0:
                            eng.tensor_copy(pacc[j][:], T[j][:])
                        else:
                            eng.tensor_add(pacc[j][:], pacc[j][:], T[j][:])
                        nc.tensor.matmul(
                            cps[j][:],
                            vsb[kt0][:, h * 128 : (h + 1) * 128],
                            T[j][:, 0:512],
                            start=(kt0 == 0), stop=False,
                        )
                        nc.tensor.matmul(
                            cps[j][:],
                            vsb[kt1][:, h * 128 : (h + 1) * 128],
                            T[j][:, 512:1024],
                            start=False, stop=False,
                        )
                # diagonal: merged pairs (ki0,ki2) then (ki1,ki3)
                kd = nkt - 4
                if nkt == 4:
                    for j, h in ((0, h0), (1, h1)):
                        nc.vector.memset(pacc[j][:], 0.0)
                T02 = [None, None]
                T13 = [None, None]
                for j, h in ((0, h0), (1, h1)):
                    sp = ps.tile([128, 1024], F32, tag="ps", name="s_ps")
                    nc.tensor.matmul(
                        sp[:, 0:512],
                        kT[h][:, kd * 128 : (kd + 1) * 128],
                        qT[h][:, qbase : qbase + 512],
                        start=True, stop=True,
                    )
                    nc.tensor.matmul(
                        sp[:, 512:768],
                        kT[h][:, (kd + 2) * 128 : (kd + 3) * 128],
                        qT[h][:, qbase + 256 : qbase + 512],
                        start=True, stop=True,
                    )
                    T02[j] = ptp.tile([128, 1024], BF16, tag="pT", name="pT")
                    nc.scalar.activation(
                        T02[j][:, 0:768], sp[:, 0:768], EXP, scale=float(SCALE)
                    )
                for j, h in ((0, h0), (1, h1)):
                    eng = acc_eng(h)
                    eng.tensor_mul(T02[j][:, 0:128], T02[j][:, 0:128], tri[:])
                    eng.tensor_mul(T02[j][:, 512:640], T02[j][:, 512:640], tri[:])
                    eng.tensor_add(
                        pacc[j][:, 0:512], pacc[j][:, 0:512], T02[j][:, 0:512]
                    )
                    eng.tensor_add(
                        pacc[j][:, 768:1024], pacc[j][:, 768:1024], T02[j][:, 512:768]
                    )
                    nc.tensor.matmul(
                        cps[j][:],
                        vsb[kd][:, h * 128 : (h + 1) * 128],
                        T02[j][:, 0:512],
                        start=(kd == 0), stop=False,
                    )
                    nc.tensor.matmul(
                        cps[j][:, 256:512],
                        vsb[kd + 2][:, h * 128 : (h + 1) * 128],
                        T02[j][:, 512:768],
                        start=False, stop=False,
                    )
                for j, h in ((0, h0), (1, h1)):
                    # ki1 [0:384] and ki3 [384:512] pack into ONE psum bank;
                    # ki3 uses start=False so the bank's has_written bits from
                    # ki1 are preserved (ki3's region was unwritten -> plain
                    # overwrite) and one 512-wide exp covers both
                    sp = ps.tile([128, 1024], F32, tag="ps", name="s_ps")
                    nc.tensor.matmul(
                        sp[:, 0:384],
                        kT[h][:, (kd + 1) * 128 : (kd + 2) * 128],
                        qT[h][:, qbase + 128 : qbase + 512],
                        start=True, stop=False,
                    )
                    nc.tensor.matmul(
                        sp[:, 384:512],
                        kT[h][:, (kd + 3) * 128 : (kd + 4) * 128],
                        qT[h][:, qbase + 384 : qbase + 512],
                        start=False, stop=True,
                    )
                    T13[j] = ptp.tile([128, 1024], BF16, tag="pT", name="pT")
                    nc.scalar.activation(
                        T13[j][:, 0:512], sp[:, 0:512], EXP, scale=float(SCALE)
                    )
                for j, h in ((0, h0), (1, h1)):
                    eng = acc_eng(h)
                    eng.tensor_mul(T13[j][:, 0:128], T13[j][:, 0:128], tri[:])
                    eng.tensor_mul(T13[j][:, 384:512], T13[j][:, 384:512], tri[:])
                    eng.tensor_add(
                        pacc[j][:, 640:1024], pacc[j][:, 640:1024], T13[j][:, 0:384]
                    )
                    eng.tensor_add(
                        pacc[j][:, 896:1024], pacc[j][:, 896:1024], T13[j][:, 384:512]
                    )
                    nc.tensor.matmul(
                        cps[j][:, 128:512],
                        vsb[kd + 1][:, h * 128 : (h + 1) * 128],
                        T13[j][:, 0:384],
                        start=False, stop=False,
                    )
                    nc.tensor.matmul(
                        cps[j][:, 384:512],
                        vsb[kd + 3][:, h * 128 : (h + 1) * 128],
                        T13[j][:, 384:512],
                        start=False, stop=True,
                    )
                if hp == 0:
                    flush_epilogue()
                for j, h in ((0, h0), (1, h1)):
                    nc.tensor.matmul(
                        row_ps[:], e8[:, h * HPC : (h + 1) * HPC], pacc[j][:, 0:512],
                        start=not row_started[0], stop=False,
                    )
                    row_started[0] = True
                    nc.tensor.matmul(
                        row_ps[:], e8[:, h * HPC : (h + 1) * HPC],
                        pacc[j][:, 512:1024],
                        start=False, stop=(hp == 3 and j == 1),
                    )
                    hc = (qb % 2) * HPC + h
                    nc.vector.tensor_copy(
                        ctx_all[:, hc * 512 : (hc + 1) * 512], cps[j][:]
                    )

            # reciprocal emitted eagerly (DVE runs it while the PE is still
            # deep in this qb's tail / next qb's head); the bc/normalize/
            # project chain is deferred into the next qb's first pair
            recip = rcp.tile([HPC, 512], BF16, tag="recip", name="recip")
            nc.vector.reciprocal(recip[:], row_ps[:])

            def make_epilogue(qb=qb, qbase=qbase, recip=recip, out_ps=out_ps):
                def epi():
                    # broadcast 1/r for all 8 heads first (only gated on
                    # recip, so the PE can run them back-to-back); the
                    # mult->project pairs then trickle in as DVE finishes
                    # each normalize, never blocking the PE queue head
                    bb = [None] * HPC
                    for pair2 in range(4):
                        for h in (2 * pair2, 2 * pair2 + 1):
                            bb[h] = ps.tile([128, 512], F32, tag="ps", name="bc_ps")
                            nc.tensor.matmul(
                                bb[h][:], e2[:, h * 128 : (h + 1) * 128], recip[:],
                                start=True, stop=True,
                            )
                        for h in (2 * pair2, 2 * pair2 + 1):
                            hc = (qb % 2) * HPC + h
                            ctxn = csp.tile([128, 512], BF16, tag="ctxn", name="ctxn")
                            nc.vector.tensor_mul(
                                ctxn[:], ctx_all[:, hc * 512 : (hc + 1) * 512],
                                bb[h][:],
                            )
                            nc.tensor.matmul(
                                out_ps[:], wo[:, h * 128 : (h + 1) * 128], ctxn[:],
                                start=(h == 0), stop=(h == HPC - 1),
                            )
                    nc.vector.tensor_copy(out_acc[:, qbase : qbase + 512], out_ps[:])
                return epi

            pending_epilogue[0] = make_epilogue()

        flush_epilogue()
        nc.sync.dma_start(out_d[:], out_acc[:])

    nc.compile()
    return nc


def _get_nc():
    if "nc" not in _CACHE:
        _CACHE["nc"] = _build_nc()
    return _CACHE["nc"]


def shard_inputs(query, Wq, bq, Wk, bk, Wv, bv, Wo, bo=None):
    import ml_dtypes

    BF = ml_dtypes.bfloat16
    query = np.asarray(query, np.float32)
    Wq, bq = np.asarray(Wq, np.float32), np.asarray(bq, np.float32)
    Wk = np.asarray(Wk, np.float32)
    Wv = np.asarray(Wv, np.float32)
    Wo = np.asarray(Wo, np.float32)

    kk = np.arange(128)[:, None]
    tri = (kk <= np.arange(128)[None, :]).astype(BF)  # [k, q]: k<=q valid
    e8 = np.zeros((128, HPC * HPC), BF)
    for h in range(HPC):
        e8[:, h * HPC + h] = 1.0
    e2 = np.zeros((HPC, HPC * 128), BF)
    for h in range(HPC):
        e2[h, h * 128 : (h + 1) * 128] = 1.0

    in_maps = []
    for c in range(N_CORES):
        b, g = c // 2, c % 2
        hs = slice(g * HPC * 128, (g + 1) * HPC * 128)
        wo_l = (
            Wo[hs, :].reshape(HPC, 128, 128).transpose(1, 0, 2).reshape(128, HPC * 128)
        )
        in_maps.append(
            {
                "xt": np.ascontiguousarray(query[b].T).astype(BF),
                "wq": np.ascontiguousarray(Wq[:, hs]).astype(BF),
                "wk": np.ascontiguousarray(Wk[:, hs]).astype(BF),
                "wv": np.ascontiguousarray(Wv[:, hs]).astype(BF),
                "wo": np.ascontiguousarray(wo_l).astype(BF),
                "bqc": np.ascontiguousarray(bq[hs].reshape(HPC, 128).T),
                "tri": tri,
                "e8": e8,
                "e2": e2,
            }
        )
    return in_maps


def kernel(**inputs):
    _import_concourse()
    from concourse import bass_utils

    bo = np.asarray(inputs["bo"], np.float32)
    bv = np.asarray(inputs["bv"], np.float32)
    Wo = np.asarray(inputs["Wo"], np.float32)
    const_row = bo + bv @ Wo  # folded V-bias + output bias
    nc = _get_nc()
    in_maps = shard_inputs(**inputs)
    res = bass_utils.run_bass_kernel_spmd(nc, in_maps, list(range(N_CORES))).results
    out = np.empty((B, S, 128), np.float32)
    for b in range(B):
        out[b] = (res[2 * b]["out_t"] + res[2 * b + 1]["out_t"]).T + const_row
    return out


# revision 21
# speedup vs baseline: 1.5540x; 1.5540x over previous
"""Multi-head causal self-attention (B=4, S=2048, H=16, D=128) on 8 TRN2 cores.

Sharding: core c = (batch b = c//2, head-group g = c%2 of 8 heads).
Each core computes Q/K projections (bf16) for its 8 heads, V, causal
attention, and the partial output projection. Host sums the two head-group
partials per batch and adds bo + bv@Wo (see bias folds below).

Bias folds (exact):
 - K bias is dropped: softmax_k[(K+bk)·(Q+bq)] == softmax_k[K·(Q+bq)]
   (terms constant along k cancel).
 - V bias and bo are folded into the host gather: normalized probs sum to
   1, so ctx_norm = ctx0/r + bv, whose projection is the constant row
   vector bv@Wo added host-side.

Structure (all matmul operands bf16; PSUM fp32):
 - kT/qT per head: [128 d, 2048 s]; V: 16 s-chunks of [128 s, 8*128 d].
 - k-tiles are processed in PAIRS: two score matmuls fill the two banks of
   one [128,1024] PSUM tile, ONE ACT exp covers both (amortizes the
   per-instruction overhead), writing a [128,1024] bf16 probs tile.
 - Softmax denominators WITHOUT per-tile ones-matmuls: a per-(head,qblock)
   running sum pacc [128,1024] accumulates the probs tiles on DVE (even
   heads) / GPSIMD (odd heads) -- engines that are otherwise idle -- and a
   single pair of E8 one-hot matmuls per (head,qblock) reduces pacc into
   the shared [8,512] row PSUM tile (head h lands on partition h). One DVE
   reciprocal per qblock serves all 8 heads.
 - Causal diagonal: triangular streaming (q starts at the k-tile diagonal)
   + one [128,128] bf16 mask multiply per diagonal subtile. Diagonal
   subtiles are exp-merged as (ki0,ki2) and (ki1,ki3) pairs.
 - Normalization: broadcast 1/r via one-hot-row E2 matmul, one DVE
   multiply, out-projection accumulates all 8 heads in one PSUM bank.
 - The per-qb epilogue is EMITTED one head-pair into the next q-block so
   the PE never idles at the boundary (HAM stays warm); ctx staging is
   double-buffered by qb parity to keep that legal.

q-blocks run in ASCENDING size order: the small early blocks overlap the
phase-1 drain, every deferred epilogue lands in a BIGGER next block, and
the dense qb=3 stream finishes the kernel with the PE warm."""

import os
import sys

import numpy as np

NUM_HEADS = 16
D = 128
B = 4
S = 2048
HPC = 8  # heads per core
N_CORES = 8
SCALE = 1.0 / np.sqrt(128.0)

_CACHE = {}


def _import_concourse():
    if "/opt/trn_rl_repo" not in sys.path and os.path.isdir("/opt/trn_rl_repo"):
        sys.path.insert(0, "/opt/trn_rl_repo")


def _build_nc():
    _import_concourse()
    from contextlib import ExitStack

    import concourse.mybir as mybir
    import concourse.tile as tile
    from concourse import bacc

    F32 = mybir.dt.float32
    BF16 = mybir.dt.bfloat16
    EXP = mybir.ActivationFunctionType.Exp

    nc = bacc.Bacc(trn_type="TRN2", target_bir_lowering=False, debug=False)

    xt_d = nc.dram_tensor("xt", [128, S], BF16, kind="ExternalInput").ap()
    wq_d = nc.dram_tensor("wq", [128, HPC * 128], BF16, kind="ExternalInput").ap()
    wk_d = nc.dram_tensor("wk", [128, HPC * 128], BF16, kind="ExternalInput").ap()
    wv_d = nc.dram_tensor("wv", [128, HPC * 128], BF16, kind="ExternalInput").ap()
    wo_d = nc.dram_tensor("wo", [128, HPC * 128], BF16, kind="ExternalInput").ap()
    bq_d = nc.dram_tensor("bqc", [128, HPC], F32, kind="ExternalInput").ap()
    tri_d = nc.dram_tensor("tri", [128, 128], BF16, kind="ExternalInput").ap()
    e8_d = nc.dram_tensor("e8", [128, HPC * HPC], BF16, kind="ExternalInput").ap()
    e2_d = nc.dram_tensor("e2", [HPC, HPC * 128], BF16, kind="ExternalInput").ap()
    out_d = nc.dram_tensor("out_t", [128, S], F32, kind="ExternalOutput").ap()

    with ExitStack() as ctx:
        ctx.enter_context(
            nc.allow_low_precision(reason="bf16 attention, tol 2e-2 rel")
        )
        tc = ctx.enter_context(tile.TileContext(nc))
        sb = ctx.enter_context(tc.tile_pool(name="sb", bufs=1))
        ptp = ctx.enter_context(tc.tile_pool(name="ptp", bufs=8))
        pap = ctx.enter_context(tc.tile_pool(name="pap", bufs=6))
        csp = ctx.enter_context(tc.tile_pool(name="csp", bufs=3))
        rcp = ctx.enter_context(tc.tile_pool(name="rcp", bufs=2))
        ps = ctx.enter_context(tc.tile_pool(name="ps", bufs=2, space="PSUM"))
        psc = ctx.enter_context(tc.tile_pool(name="psc", bufs=2, space="PSUM"))
        psr = ctx.enter_context(tc.tile_pool(name="psr", bufs=1, space="PSUM"))
        pso = ctx.enter_context(tc.tile_pool(name="pso", bufs=1, space="PSUM"))

        def load(name, dram_ap, shape, dt):
            t = sb.tile(shape, dt, tag=name, name=name)
            nc.sync.dma_start(t[:], dram_ap[:])
            return t

        xt = load("xt", xt_d, [128, S], BF16)
        wq = load("wq", wq_d, [128, HPC * 128], BF16)
        wk = load("wk", wk_d, [128, HPC * 128], BF16)
        wv = load("wv", wv_d, [128, HPC * 128], BF16)
        wo = load("wo", wo_d, [128, HPC * 128], BF16)
        bqc = load("bqc", bq_d, [128, HPC], F32)
        tri = load("tri", tri_d, [128, 128], BF16)
        e8 = load("e8", e8_d, [128, HPC * HPC], BF16)
        e2 = load("e2", e2_d, [HPC, HPC * 128], BF16)

        out_acc = sb.tile([128, S], F32, tag="out_acc")
        # ctx staging double-buffered by qb parity (the deferred epilogue of
        # qb reads while qb-1's pairs write the other half)
        ctx_all = sb.tile([128, 2 * HPC * 512], BF16, tag="ctx_all")

        # ---- phase 1: projections ----
        kT = [sb.tile([128, S], BF16, tag=f"kT{h}", name=f"kT{h}") for h in range(HPC)]
        qT = [sb.tile([128, S], BF16, tag=f"qT{h}", name=f"qT{h}") for h in range(HPC)]
        vsb = [
            sb.tile([128, HPC * 128], BF16, tag=f"v{st}", name=f"v{st}")
            for st in range(16)
        ]

        def proj_head(h):
            # 1024-wide PSUM staging (2 banks, 2 matmuls) with one wide
            # drain copy -- K on DVE, Q on ACT (bias fused) -- so the drain
            # keeps pace with the PE and HAM warms up early
            hs = slice(h * 128, (h + 1) * 128)
            for s2 in range(2):
                sl = slice(s2 * 1024, (s2 + 1) * 1024)
                psK = ps.tile([128, 1024], F32, tag="ps", name="psK")
                for half in range(2):
                    xsl = slice(s2 * 1024 + half * 512, s2 * 1024 + (half + 1) * 512)
                    nc.tensor.matmul(
                        psK[:, half * 512 : (half + 1) * 512], wk[:, hs], xt[:, xsl],
                        start=True, stop=True,
                    )
                nc.vector.tensor_copy(kT[h][:, sl], psK[:])
                psQ = ps.tile([128, 1024], F32, tag="ps", name="psQ")
                for half in range(2):
                    xsl = slice(s2 * 1024 + half * 512, s2 * 1024 + (half + 1) * 512)
                    nc.tensor.matmul(
                        psQ[:, half * 512 : (half + 1) * 512], wq[:, hs], xt[:, xsl],
                        start=True, stop=True,
                    )
                nc.scalar.activation(
                    qT[h][:, sl], psQ[:],
                    mybir.ActivationFunctionType.Identity,
                    bias=bqc[:, h : h + 1], scale=1.0,
                )

        def proj_v():
            for st in range(16):
                xsl = slice(st * 128, (st + 1) * 128)
                psV = ps.tile([128, 1024], F32, tag="ps", name="psV")
                for j in range(2):
                    wsl = slice(j * 512, (j + 1) * 512)
                    nc.tensor.matmul(
                        psV[:, wsl], xt[:, xsl], wv[:, wsl], start=True, stop=True
                    )
                if st % 2 == 0:
                    nc.vector.tensor_copy(vsb[st][:], psV[:])
                else:
                    nc.scalar.copy(vsb[st][:], psV[:])

        proj_head(0)
        proj_head(1)
        proj_v()
        for h in range(2, HPC):
            proj_head(h)

        # ---- phase 2: attention, q-blocks descending ----
        pending_epilogue = [None]

        def flush_epilogue():
            if pending_epilogue[0] is not None:
                pending_epilogue[0]()
                pending_epilogue[0] = None

        def acc_eng(h):
            # pacc/mask engine: DVE for even heads, GPSIMD for odd
            return nc.vector

        for qb in (0, 1, 2, 3):
            nkt = 4 * (qb + 1)
            qbase = qb * 512
            row_ps = psr.tile([HPC, 512], F32, tag="row", name="row_ps")
            out_ps = pso.tile([128, 512], F32, tag="o", name="out_ps")
            row_started = [False]
            for hp in range(4):
                h0, h1 = 2 * hp, 2 * hp + 1
                cps = [
                    psc.tile([128, 512], F32, tag="ctx", name="ctx_ps")
                    for _ in range(2)
                ]
                pacc = [
                    pap.tile([128, 1024], BF16, tag="pacc", name="pacc")
                    for _ in range(2)
                ]
                # non-diagonal k-tile pairs
                for p in range((nkt - 4) // 2):
                    kt0, kt1 = 2 * p, 2 * p + 1
                    T = [None, None]
                    for j, h in ((0, h0), (1, h1)):
                        sp = ps.tile([128, 1024], F32, tag="ps", name="s_ps")
                        nc.tensor.matmul(
                            sp[:, 0:512],
                            kT[h][:, kt0 * 128 : (kt0 + 1) * 128],
                            qT[h][:, qbase : qbase + 512],
                            start=True, stop=True,
                        )
                        nc.tensor.matmul(
                            sp[:, 512:1024],
                            kT[h][:, kt1 * 128 : (kt1 + 1) * 128],
                            qT[h][:, qbase : qbase + 512],
                            start=True, stop=True,
                        )
                        T[j] = ptp.tile([128, 1024], BF16, tag="pT", name="pT")
                        nc.scalar.activation(T[j][:], sp[:], EXP, scale=float(SCALE))
                    for j, h in ((0, h0), (1, h1)):
                        eng = acc_eng(h)
                        if p == 0:
                            eng.tensor_copy(pacc[j][:], T[j][:])
                        else:
                            eng.tensor_add(pacc[j][:], pacc[j][:], T[j][:])
                        nc.tensor.matmul(
                            cps[j][:],
                            vsb[kt0][:, h * 128 : (h + 1) * 128],
                            T[j][:, 0:512],
                            start=(kt0 == 0), stop=False,
                        )
                        nc.tensor.matmul(
                            cps[j][:],
                            vsb[kt1][:, h * 128 : (h + 1) * 128],
                            T[j][:, 512:1024],
                            start=False, stop=False,
                        )
                # diagonal: merged pairs (ki0,ki2) then (ki1,ki3)
                kd = nkt - 4
                if nkt == 4:
                    for j, h in ((0, h0), (1, h1)):
                        nc.vector.memset(pacc[j][:], 0.0)
                T02 = [None, None]
                T13 = [None, None]
                for j, h in ((0, h0), (1, h1)):
                    sp = ps.tile([128, 1024], F32, tag="ps", name="s_ps")
                    nc.tensor.matmul(
                        sp[:, 0:512],
                        kT[h][:, kd * 128 : (kd + 1) * 128],
                        qT[h][:, qbase : qbase + 512],
                        start=True, stop=True,
                    )
                    nc.tensor.matmul(
                        sp[:, 512:768],
                        kT[h][:, (kd + 2) * 128 : (kd + 3) * 128],
                        qT[h][:, qbase + 256 : qbase + 512],
                        start=True, stop=True,
                    )
                    T02[j] = ptp.tile([128, 1024], BF16, tag="pT", name="pT")
                    nc.scalar.activation(
                        T02[j][:, 0:768], sp[:, 0:768], EXP, scale=float(SCALE)
                    )
                for j, h in ((0, h0), (1, h1)):
                    eng = acc_eng(h)
                    eng.tensor_mul(T02[j][:, 0:128], T02[j][:, 0:128], tri[:])
                    eng.tensor_mul(T02[j][:, 512:640], T02[j][:, 512:640], tri[:])
                    eng.tensor_add(
                        pacc[j][:, 0:512], pacc[j][:, 0:512], T02[j][:, 0:512]
                    )
                    eng.tensor_add(
                        pacc[j][:, 768:1024], pacc[j][:, 768:1024], T02[j][:, 512:768]
                    )
                    nc.tensor.matmul(
                        cps[j][:],
                        vsb[kd][:, h * 128 : (h + 1) * 128],
                        T02[j][:, 0:512],
                        start=(kd == 0), stop=False,
                    )
                    nc.tensor.matmul(
                        cps[j][:, 256:512],
                        vsb[kd + 2][:, h * 128 : (h + 1) * 128],
                        T02[j][:, 512:768],
                        start=False, stop=False,
                    )
                for j, h in ((0, h0), (1, h1)):
                    # ki1 [0:384] and ki3 [384:512] pack into ONE psum bank;
                    # ki3 uses start=False so the bank's has_written bits from
                    # ki1 are preserved (ki3's region was unwritten -> plain
                    # overwrite) and one 512-wide exp covers both
                    sp = ps.tile([128, 1024], F32, tag="ps", name="s_ps")
                    nc.tensor.matmul(
                        sp[:, 0:384],
                        kT[h][:, (kd + 1) * 128 : (kd + 2) * 128],
                        qT[h][:, qbase + 128 : qbase + 512],
                        start=True, stop=False,
                    )
                    nc.tensor.matmul(
                        sp[:, 384:512],
                        kT[h][:, (kd + 3) * 128 : (kd + 4) * 128],
                        qT[h][:, qbase + 384 : qbase + 512],
                        start=False, stop=True,
                    )
                    T13[j] = ptp.tile([128, 1024], BF16, tag="pT", name="pT")
                    nc.scalar.activation(
                        T13[j][:, 0:512], sp[:, 0:512], EXP, scale=float(SCALE)
                    )
                for j, h in ((0, h0), (1, h1)):
                    eng = acc_eng(h)
                    eng.tensor_mul(T13[j][:, 0:128], T13[j][:, 0:128], tri[:])
                    eng.tensor_mul(T13[j][:, 384:512], T13[j][:, 384:512], tri[:])
                    eng.tensor_add(
                        pacc[j][:, 640:1024], pacc[j][:, 640:1024], T13[j][:, 0:384]
                    )
                    eng.tensor_add(
                        pacc[j][:, 896:1024], pacc[j][:, 896:1024], T13[j][:, 384:512]
                    )
                    nc.tensor.matmul(
                        cps[j][:, 128:512],
                        vsb[kd + 1][:, h * 128 : (h + 1) * 128],
                        T13[j][:, 0:384],
                        start=False, stop=False,
                    )
                    nc.tensor.matmul(
                        cps[j][:, 384:512],
                        vsb[kd + 3][:, h * 128 : (h + 1) * 128],
                        T13[j][:, 384:512],
                        start=False, stop=True,
                    )
                if hp == 0:
                    flush_epilogue()
                for j, h in ((0, h0), (1, h1)):
                    nc.tensor.matmul(
                        row_ps[:], e8[:, h * HPC : (h + 1) * HPC], pacc[j][:, 0:512],
                        start=not row_started[0], stop=False,
                    )
                    row_started[0] = True
                    nc.tensor.matmul(
                        row_ps[:], e8[:, h * HPC : (h + 1) * HPC],
                        pacc[j][:, 512:1024],
                        start=False, stop=(hp == 3 and j == 1),
                    )
                    hc = (qb % 2) * HPC + h
                    nc.scalar.copy(ctx_all[:, hc * 512 : (hc + 1) * 512], cps[j][:])

            # reciprocal emitted eagerly (DVE runs it while the PE is still
            # deep in this qb's tail / next qb's head); the bc/normalize/
            # project chain is deferred into the next qb's first pair
            recip = rcp.tile([HPC, 512], BF16, tag="recip", name="recip")
            nc.vector.reciprocal(recip[:], row_ps[:])

            def make_epilogue(qb=qb, qbase=qbase, recip=recip, out_ps=out_ps):
                def epi():
                    # broadcast 1/r for all 8 heads first (only gated on
                    # recip, so the PE can run them back-to-back); the
                    # mult->project pairs then trickle in as DVE finishes
                    # each normalize, never blocking the PE queue head
                    bb = [None] * HPC
                    for pair2 in range(4):
                        for h in (2 * pair2, 2 * pair2 + 1):
                            bb[h] = ps.tile([128, 512], F32, tag="ps", name="bc_ps")
                            nc.tensor.matmul(
                                bb[h][:], e2[:, h * 128 : (h + 1) * 128], recip[:],
                                start=True, stop=True,
                            )
                        for h in (2 * pair2, 2 * pair2 + 1):
                            hc = (qb % 2) * HPC + h
                            ctxn = csp.tile([128, 512], BF16, tag="ctxn", name="ctxn")
                            nc.vector.tensor_mul(
                                ctxn[:], ctx_all[:, hc * 512 : (hc + 1) * 512],
                                bb[h][:],
                            )
                            nc.tensor.matmul(
                                out_ps[:], wo[:, h * 128 : (h + 1) * 128], ctxn[:],
                                start=(h == 0), stop=(h == HPC - 1),
                            )
                    nc.vector.tensor_copy(out_acc[:, qbase : qbase + 512], out_ps[:])
                    nc.sync.dma_start(
                        out_d[:, qbase : qbase + 512],
                        out_acc[:, qbase : qbase + 512],
                    )
                return epi

            pending_epilogue[0] = make_epilogue()

        flush_epilogue()

    nc.compile()
    return nc


def _get_nc():
    if "nc" not in _CACHE:
        _CACHE["nc"] = _build_nc()
    return _CACHE["nc"]


def shard_inputs(query, Wq, bq, Wk, bk, Wv, bv, Wo, bo=None):
    import ml_dtypes

    BF = ml_dtypes.bfloat16
    query = np.asarray(query, np.float32)
    Wq, bq = np.asarray(Wq, np.float32), np.asarray(bq, np.float32)
    Wk = np.asarray(Wk, np.float32)
    Wv = np.asarray(Wv, np.float32)
    Wo = np.asarray(Wo, np.float32)

    kk = np.arange(128)[:, None]
    tri = (kk <= np.arange(128)[None, :]).astype(BF)  # [k, q]: k<=q valid
    e8 = np.zeros((128, HPC * HPC), BF)
    for h in range(HPC):
        e8[:, h * HPC + h] = 1.0
    e2 = np.zeros((HPC, HPC * 128), BF)
    for h in range(HPC):
        e2[h, h * 128 : (h + 1) * 128] = 1.0

    in_maps = []
    for c in range(N_CORES):
        b, g = c // 2, c % 2
        hs = slice(g * HPC * 128, (g + 1) * HPC * 128)
        wo_l = (
            Wo[hs, :].reshape(HPC, 128, 128).transpose(1, 0, 2).reshape(128, HPC * 128)
        )
        in_maps.append(
            {
                "xt": np.ascontiguousarray(query[b].T).astype(BF),
                "wq": np.ascontiguousarray(Wq[:, hs]).astype(BF),
                "wk": np.ascontiguousarray(Wk[:, hs]).astype(BF),
                "wv": np.ascontiguousarray(Wv[:, hs]).astype(BF),
                "wo": np.ascontiguousarray(wo_l).astype(BF),
                "bqc": np.ascontiguousarray(bq[hs].reshape(HPC, 128).T),
                "tri": tri,
                "e8": e8,
                "e2": e2,
            }
        )
    return in_maps


def kernel(**inputs):
    _import_concourse()
    from concourse import bass_utils

    bo = np.asarray(inputs["bo"], np.float32)
    bv = np.asarray(inputs["bv"], np.float32)
    Wo = np.asarray(inputs["Wo"], np.float32)
    const_row = bo + bv @ Wo  # folded V-bias + output bias
    nc = _get_nc()
    in_maps = shard_inputs(**inputs)
    res = bass_utils.run_bass_kernel_spmd(nc, in_maps, list(range(N_CORES))).results
    out = np.empty((B, S, 128), np.float32)
    for b in range(B):
        out[b] = (res[2 * b]["out_t"] + res[2 * b + 1]["out_t"]).T + const_row
    return out
